# revision 8
# baseline (speedup 1.0000x reference)
"""Trainium2 Bass kernel for nn_BoundingBoxMatcher (nms_detection).

Data-parallel over B across 8 NeuronCores, one batch per core. Per core the
(H,H) assignment-score matrix ingredients are computed on device and the tiny
32x32 Jonker-Volgenant assignment runs on host over the gathered results.

Device math per core/batch (N=65536 points, H=32 boxes):
    prod_{h,c}(x) = (v1-x_c)(v2-x_c) = x_c^2 - (v1+v2)x_c + v1*v2
is linear in features F = [x,y,z,x^2,y^2,z^2,1], so all 192 (h,c,side)
products per point are one small matmul of F against host-built coefficients
W. Evaluated in fp16x2 (hi+lo, 3 passes) for ~fp32 accuracy. Features are
PE-transposed so K=features lies on partitions; products land points-on-
partitions in PSUM. The true side drains through ACT Relu (coefficients
pre-scaled x256 to dodge fp16 subnormal flush; inside = sum relu == 0);
the pred side drains through a strided 3-way min tensor_reduce (coefficients
pre-scaled by -100, so logits = clip(min3, -20, 20)). sigmoid on ACT;
softplus via sp(l) = l - ln(sigmoid(l)) summed by the reduction matmul.
A per-chunk matmul with lhsT=[inside|ones] and rhs=[l|prob|ln(sig)|ones]
accumulates cross/tp/sum_inside/sum_prob/sum_l/sum_ln into one (33,128)
PSUM tile. Host assembles bce+iou+mse, masks padded boxes, runs JV.
"""

import numpy as np
from contextlib import ExitStack

import concourse.bass as bass
import concourse.bacc as bacc
import concourse.tile as tile
import concourse.mybir as mybir
from concourse.bass_utils import run_bass_kernel_spmd

F32 = mybir.dt.float32
F16 = mybir.dt.float16
AF = mybir.ActivationFunctionType
OP = mybir.AluOpType

B, N, H = 8, 65536, 32
P = 128          # partitions
C = N // P       # 512 point-columns per partition
NT = C // 16     # 32 transpose blocks
TRUE_SCALE = 256.0   # pre-scale on true-side W (guards fp16 subnormal flush)
PRED_SCALE = -100.0  # folds logits = -THETA1*prod into W

_NC_CACHE = {}


# ---------------------------------------------------------------- device build
def build_nc(dbg=False):
    nc = bacc.Bacc("TRN2", target_bir_lowering=False, debug=False, enable_asserts=True)
    pc = nc.dram_tensor("pc", [N, 9], F32, kind="ExternalInput").ap()
    wh = nc.dram_tensor("wh", [P, 4, 192], F16, kind="ExternalInput").ap()
    wl = nc.dram_tensor("wl", [P, 4, 192], F16, kind="ExternalInput").ap()
    ident = nc.dram_tensor("ident", [P, P], F16, kind="ExternalInput").ap()
    red_out = nc.dram_tensor("red", [33, 128], F32, kind="ExternalOutput").ap()
    if dbg:
        dbg_vals = nc.dram_tensor("dbg_vals", [2, P, 4, 256, 32], F16, kind="ExternalOutput").ap()
        dbg_ins = nc.dram_tensor("dbg_ins", [2, P, 256, 34], F16, kind="ExternalOutput").ap()
        dbg_ftb = nc.dram_tensor("dbg_ftb", [2, P, P], F16, kind="ExternalOutput").ap()

    with tile.TileContext(nc) as tc:
        with ExitStack() as ctx:
            sing = ctx.enter_context(tc.tile_pool(name="sing", bufs=1))
            fpool = ctx.enter_context(tc.tile_pool(name="fpool", bufs=1))
            half_pool = ctx.enter_context(tc.tile_pool(name="half", bufs=1))
            ftb_pool = ctx.enter_context(tc.tile_pool(name="ftb", bufs=4))
            rt_pool = ctx.enter_context(tc.tile_pool(name="rt", bufs=2))
            mp_pool = ctx.enter_context(tc.tile_pool(name="mp", bufs=2))
            sa_pool = ctx.enter_context(tc.tile_pool(name="sa", bufs=2))
            ps_tr = ctx.enter_context(tc.tile_pool(name="ps_tr", bufs=2, space="PSUM"))
            ps_prod = ctx.enter_context(tc.tile_pool(name="ps_prod", bufs=2, space="PSUM"))
            ps_red = ctx.enter_context(tc.tile_pool(name="ps_red", bufs=1, space="PSUM"))

            # ---- load inputs
            t_pc = sing.tile([P, C, 9], F32)
            nc.sync.dma_start(t_pc[:], pc.rearrange("(p c) f -> p c f", p=P))
            t_wh = sing.tile([P, 4, 192], F16)
            nc.gpsimd.dma_start(t_wh[:], wh[:])
            t_wl = sing.tile([P, 4, 192], F16)
            nc.gpsimd.dma_start(t_wl[:], wl[:])
            t_id = sing.tile([P, P], F16)
            nc.gpsimd.dma_start(t_id[:], ident[:])

            # ---- features: F8h/F8l (P, C, 8) = fp16 hi/lo of [x,y,z,x2,y2,z2,1,0]
            f8h = fpool.tile([P, C, 8], F16)
            f8l = fpool.tile([P, C, 8], F16)
            sq32 = fpool.tile([P, C, 3], F32)
            nc.scalar.copy(f8h[:, :, 0:3], t_pc[:, :, 0:3])
            nc.vector.tensor_tensor(out=f8l[:, :, 0:3], in0=t_pc[:, :, 0:3],
                                    in1=f8h[:, :, 0:3], op=OP.subtract)
            nc.scalar.square(sq32[:], t_pc[:, :, 0:3])
            nc.scalar.copy(f8h[:, :, 3:6], sq32[:])
            nc.vector.tensor_tensor(out=f8l[:, :, 3:6], in0=sq32[:],
                                    in1=f8h[:, :, 3:6], op=OP.subtract)
            nc.gpsimd.memset(f8h[:, :, 6], 1.0)
            nc.gpsimd.memset(f8h[:, :, 7], 0.0)
            nc.gpsimd.memset(f8l[:, :, 6:8], 0.0)

            # ---- persistent reduction accumulator
            red_ps = ps_red.tile([33, 128], F32)
            first_mm = True

            for half in range(2):
                nhc = 256  # chunks per half
                vals = half_pool.tile([P, 4, nhc, 32], F16, tag="vals")
                insv = half_pool.tile([P, nhc, 34], F16, tag="ins")
                sig32 = half_pool.tile([P, nhc, 32], F32, tag="sig")
                nc.gpsimd.memset(vals[:, 3], 1.0)
                nc.gpsimd.memset(insv[:, :, 32], 1.0)
                nc.gpsimd.memset(insv[:, :, 33], 0.0)

                for g in range(4):  # 4 groups x 64 chunks per half
                    rt_t = rt_pool.tile([P, 64, 96], F16, tag="rt")
                    mp_t = mp_pool.tile([P, 64, 32], F16, tag="mp")
                    for tt in range(4):
                        t = half * 16 + g * 4 + tt
                        ftbh = ftb_pool.tile([P, P], F16, tag="ftbh")
                        ftbl = ftb_pool.tile([P, P], F16, tag="ftbl")
                        p_trh = ps_tr.tile([P, P], F16, tag="tr")
                        nc.tensor.transpose(p_trh[:], f8h[:, 16 * t:16 * (t + 1), :], t_id[:])
                        nc.vector.tensor_copy(ftbh[:], p_trh[:])
                        p_trl = ps_tr.tile([P, P], F16, tag="tr")
                        nc.tensor.transpose(p_trl[:], f8l[:, 16 * t:16 * (t + 1), :], t_id[:])
                        nc.vector.tensor_copy(ftbl[:], p_trl[:])
                        if dbg and t in (0, 16):
                            nc.gpsimd.dma_start(dbg_ftb[t // 16], ftbh[:])

                        for a in range(4):
                            prod = ps_prod.tile([P, 4, 256], F32, tag="prod")
                            lh = ftbh[32 * a:32 * (a + 1), :]
                            ll = ftbl[32 * a:32 * (a + 1), :]
                            for b in range(4):
                                # start=True zeroes the whole 2KB zero-region
                                # (bank); only bank-initial slices may set it,
                                # else they wipe the sibling band's partials.
                                nc.tensor.matmul(prod[:, b, 0:192], lh,
                                                 t_wh[32 * a:32 * (a + 1), b, :],
                                                 start=(b % 2 == 0), stop=False,
                                                 tile_position=(32 * a, 0))
                            for b in range(4):
                                nc.tensor.matmul(prod[:, b, 0:192], ll,
                                                 t_wh[32 * a:32 * (a + 1), b, :],
                                                 start=False, stop=False,
                                                 tile_position=(32 * a, 0))
                            for b in range(4):
                                nc.tensor.matmul(prod[:, b, 0:192], lh,
                                                 t_wl[32 * a:32 * (a + 1), b, :],
                                                 start=False, stop=True,
                                                 tile_position=(32 * a, 0))
                            s0 = tt * 16 + a * 4
                            # ACT relu drain (true side, 4 chunks)
                            relu_in = bass.AP(tensor=prod.tensor, offset=prod.offset,
                                              ap=[prod.ap[0], [256, 4], [1, 96]])
                            nc.scalar.activation(rt_t[:, s0:s0 + 4, :], relu_in, AF.Relu)
                            # DVE min-pool drain (pred side; W prescaled by -100)
                            pool_in = bass.AP(tensor=prod.tensor, offset=prod.offset + 96,
                                              ap=[prod.ap[0], [256, 4], [1, 32], [32, 3]])
                            nc.vector.tensor_reduce(out=mp_t[:, s0:s0 + 4, :], in_=pool_in,
                                                    axis=mybir.AxisListType.X, op=OP.min)

                    # ---- group postprocessing (64 chunks)
                    c0 = g * 64
                    sa = sa_pool.tile([P, 64, 32], F16, tag="sa")
                    nc.vector.tensor_tensor(out=sa[:], in0=rt_t[:, :, 0:32],
                                            in1=rt_t[:, :, 32:64], op=OP.add)
                    # inside = (sa + rz <= 0)  <=>  (-sa >= rz)
                    nc.vector.scalar_tensor_tensor(
                        out=insv[:, c0:c0 + 64, 0:32], in0=sa[:], scalar=-1.0,
                        in1=rt_t[:, :, 64:96], op0=OP.mult, op1=OP.is_ge)
                    # logits = clip(min3, -20, 20)
                    lg = sa_pool.tile([P, 64, 32], F16, tag="lg")
                    nc.vector.tensor_scalar(out=lg[:], in0=mp_t[:], scalar1=-20.0,
                                            scalar2=20.0, op0=OP.max, op1=OP.min)
                    nc.vector.tensor_copy(vals[:, 0, c0:c0 + 64, :], lg[:])
                    nc.scalar.activation(sig32[:, c0:c0 + 64, :], lg[:], AF.Sigmoid)
                    nc.vector.tensor_copy(vals[:, 1, c0:c0 + 64, :], sig32[:, c0:c0 + 64, :])

                # ---- per half: ln(sigmoid) (one table-set switch), then reductions
                nc.scalar.activation(vals[:, 2], sig32[:], AF.Ln)
                if dbg:
                    nc.gpsimd.dma_start(dbg_vals[half], vals[:])
                    nc.gpsimd.dma_start(dbg_ins[half], insv[:])
                for j in range(nhc):
                    lhsT = insv[:, j, 0:33]
                    rhs = bass.AP(tensor=vals.tensor, offset=vals.offset + j * 32,
                                  ap=[vals.ap[0], [nhc * 32, 4], [1, 32]])
                    nc.tensor.matmul(red_ps[:], lhsT, rhs,
                                     start=first_mm, stop=(half == 1 and j == nhc - 1))
                    first_mm = False

            s_red = sing.tile([33, 128], F32)
            nc.vector.tensor_copy(s_red[:], red_ps[:])
            nc.sync.dma_start(red_out[:], s_red[:])

    nc.compile()
    return nc


# ---------------------------------------------------------------- host helpers
def _host_w(vt_b, vp_b):
    """(128, 4, 192) fp16 hi/lo coefficient tiles for one batch.

    Column j = side*96 + coord*32 + h (side 0 = true x256, 1 = pred x-100).
    Row 32a+k active for band b=k//8 only; feature f=k%8 of
    [x,y,z,x2,y2,z2,1,pad].
    """
    w7 = np.zeros((7, 192), dtype=np.float64)
    for side, (v, scale) in enumerate([(vt_b, TRUE_SCALE), (vp_b, PRED_SCALE)]):
        v1 = v[:, 0, :].astype(np.float64)
        v2 = v[:, 1, :].astype(np.float64)
        for c in range(3):
            base = side * 96 + c * 32
            w7[c, base:base + 32] = -(v1[:, c] + v2[:, c]) * scale
            w7[3 + c, base:base + 32] = 1.0 * scale
            w7[6, base:base + 32] = v1[:, c] * v2[:, c] * scale
    w8 = np.zeros((8, 192), dtype=np.float64)
    w8[:7] = w7
    whi = w8.astype(np.float16)
    wlo = (w8 - whi.astype(np.float64)).astype(np.float16)

    def expand(w):
        full = np.zeros((P, 4, 192), dtype=np.float16)
        for a in range(4):
            for b in range(4):
                full[32 * a + 8 * b:32 * a + 8 * b + 8, b, :] = w
        return full
    return expand(whi), expand(wlo)


def _lsa(cost):
    """Jonker-Volgenant shortest augmenting path; matches scipy col_ind."""
    cost = np.asarray(cost, dtype=np.float64)
    n = cost.shape[0]
    INF = np.inf
    u = np.zeros(n + 1)
    v = np.zeros(n + 1)
    p = np.zeros(n + 1, dtype=np.int64)
    way = np.zeros(n + 1, dtype=np.int64)
    for i in range(1, n + 1):
        p[0] = i
        j0 = 0
        minv = np.full(n + 1, INF)
        used = np.zeros(n + 1, dtype=bool)
        while True:
            used[j0] = True
            i0 = p[j0]
            cur = cost[i0 - 1, :] - u[i0] - v[1:]
            upd = (~used[1:]) & (cur < minv[1:])
            minv[1:][upd] = cur[upd]
            way[1:][upd] = j0
            free = ~used[1:]
            j1 = int(np.argmin(np.where(free, minv[1:], INF))) + 1
            delta = minv[j1]
            u[p[used]] += delta
            v[used] -= delta
            minv[~used] -= delta
            j0 = j1
            if p[j0] == 0:
                break
        while j0:
            j1 = way[j0]
            p[j0] = p[j1]
            j0 = j1
    ans = np.zeros(n, dtype=np.int64)
    for j in range(1, n + 1):
        ans[p[j] - 1] = j - 1
    return ans


def assemble_score(red, vertices_true, vertices_pred):
    """red: (B, 33, 128) device outputs -> score (B, 32, 32) float32."""
    score = np.zeros((B, H, H), dtype=np.float64)
    vt = np.asarray(vertices_true, dtype=np.float64)
    vp = np.asarray(vertices_pred, dtype=np.float64)
    for b in range(B):
        r = red[b].astype(np.float64)
        cross_n = r[0:32, 0:32]          # sum_n inside*l
        tp = r[0:32, 32:64]
        sum_inside = r[0:32, 96]
        sum_l = r[32, 0:32]
        sum_prob = r[32, 32:64]
        sum_ln = r[32, 64:96]
        sum_sp = sum_l - sum_ln          # softplus(l) = l - ln(sigmoid(l))
        bce = sum_sp[None, :] / N - cross_n / N
        denom = sum_prob[None, :] + sum_inside[:, None] - tp
        with np.errstate(divide="ignore", invalid="ignore"):
            iou = -np.where(denom == 0.0, 0.0, tp / denom)
        mse = np.mean((vt[b][:, None] - vp[b][None]) ** 2, axis=(2, 3))
        score[b] = bce + iou + mse
    return score.astype(np.float32)


def hungarian(score, vertices_true):
    lm = np.asarray(score)
    vt = np.asarray(vertices_true)
    mask = np.sum(np.abs(vt), axis=(2, 3)) == 0.0
    max_loss = 2.0 * lm.max()
    lm = np.where(mask[:, :, None], max_loss, lm)
    return np.stack([_lsa(lm[b]) for b in range(lm.shape[0])], axis=0).astype(np.int32)


def make_in_maps(pointcloud, vertices_true, vertices_pred):
    ident = np.eye(P, dtype=np.float16)
    in_maps = []
    for b in range(B):
        whi, wlo = _host_w(np.asarray(vertices_true)[b], np.asarray(vertices_pred)[b])
        in_maps.append({
            "pc": np.ascontiguousarray(np.asarray(pointcloud)[b], dtype=np.float32),
            "wh": whi, "wl": wlo, "ident": ident,
        })
    return in_maps


def get_nc():
    if "nc" not in _NC_CACHE:
        _NC_CACHE["nc"] = build_nc()
    return _NC_CACHE["nc"]


def run_device(pointcloud, vertices_true, vertices_pred, trace=False):
    nc = get_nc()
    in_maps = make_in_maps(pointcloud, vertices_true, vertices_pred)
    res = run_bass_kernel_spmd(nc, in_maps, core_ids=list(range(B)), trace=trace)
    red = np.stack([res.results[i]["red"] for i in range(B)], axis=0)
    return red, res


def kernel(pointcloud, vertices_true, vertices_pred):
    red, _ = run_device(pointcloud, vertices_true, vertices_pred, trace=False)
    score = assemble_score(red, vertices_true, vertices_pred)
    return hungarian(score, vertices_true)


# revision 14
# speedup vs baseline: 1.0522x; 1.0522x over previous
"""Trainium2 Bass kernel for nn_BoundingBoxMatcher (nms_detection).

Data-parallel over B across 8 NeuronCores, one batch per core. Per core the
(H,H) assignment-score matrix ingredients are computed on device and the tiny
32x32 Jonker-Volgenant assignment runs on host over the gathered results.

Device math per core/batch (N=65536 points, H=32 boxes):
    prod_{h,c}(x) = (v1-x_c)(v2-x_c) = x_c^2 - (v1+v2)x_c + v1*v2
is linear in features F = [x,y,z,x^2,y^2,z^2,1], so all 192 (h,c,side)
products of one point are a single small matmul of F against host-built
coefficients W. fp32-grade accuracy comes from an fp16x2 split evaluated in
ONE K=24 matmul per 128-point chunk: K rows = [F_hi(8) | F_lo(8) | F_hi(8)]
against [W_hi | W_hi | W_lo], accumulating the three cross terms in PSUM.
Features are PE-transposed (128 transposes of (128,128)) so K sits on
partitions; the four 32-row strips of each transpose block are four chunks
whose product matmuls interleave across PE row-groups. True-side products
(pre-scaled x256 to dodge fp16 subnormal flush) drain through ACT Relu;
inside = (relu_x+relu_y+relu_z == 0). Pred-side products (pre-scaled by
-100) drain through a strided 3-way min tensor_reduce; logits = clip(min3).
sigmoid on ACT; softplus summed via sp(l) = l - ln(sigmoid(l)). A per-chunk
matmul with lhsT=[inside|ones] and rhs=[l|prob|ln(sig)|ones] accumulates
cross/tp/sum_inside/sum_prob/sum_l/sum_ln into one persistent (33,128) PSUM
tile. Host assembles bce+iou+mse, masks padded boxes, runs JV assignment.
"""

import numpy as np
from contextlib import ExitStack

import concourse.bass as bass
import concourse.bacc as bacc
import concourse.tile as tile
import concourse.mybir as mybir
from concourse.bass_utils import run_bass_kernel_spmd

F32 = mybir.dt.float32
F16 = mybir.dt.float16
AF = mybir.ActivationFunctionType
OP = mybir.AluOpType

B, N, H = 8, 65536, 32
P = 128          # partitions
C = N // P       # 512 point-columns per partition
NT = C // 4      # 128 transpose blocks (4 point-columns each)
TRUE_SCALE = 256.0   # pre-scale on true-side W (guards fp16 subnormal flush)
PRED_SCALE = -100.0  # folds logits = -THETA1*prod into W

_NC_CACHE = {}


# ---------------------------------------------------------------- device build
def build_nc(dbg=False):
    nc = bacc.Bacc("TRN2", target_bir_lowering=False, debug=False, enable_asserts=True)
    pc = nc.dram_tensor("pc", [N, 9], F32, kind="ExternalInput").ap()
    w_in = nc.dram_tensor("w", [P, 192], F16, kind="ExternalInput").ap()
    ident = nc.dram_tensor("ident", [P, P], F16, kind="ExternalInput").ap()
    red_out = nc.dram_tensor("red", [33, 128], F32, kind="ExternalOutput").ap()
    if dbg:
        dbg_vals = nc.dram_tensor("dbg_vals", [2, P, 4, 256, 32], F16, kind="ExternalOutput").ap()
        dbg_ins = nc.dram_tensor("dbg_ins", [2, P, 256, 34], F16, kind="ExternalOutput").ap()

    with tile.TileContext(nc) as tc:
        with ExitStack() as ctx:
            sing = ctx.enter_context(tc.tile_pool(name="sing", bufs=1))
            fpool = ctx.enter_context(tc.tile_pool(name="fpool", bufs=1))
            half_pool = ctx.enter_context(tc.tile_pool(name="half", bufs=1))
            ftb_pool = ctx.enter_context(tc.tile_pool(name="ftb", bufs=4))
            rt_pool = ctx.enter_context(tc.tile_pool(name="rt", bufs=1))
            mp_pool = ctx.enter_context(tc.tile_pool(name="mp", bufs=2))
            sa_pool = ctx.enter_context(tc.tile_pool(name="sa", bufs=1))
            ps_tr = ctx.enter_context(tc.tile_pool(name="ps_tr", bufs=2, space="PSUM"))
            ps_prod = ctx.enter_context(tc.tile_pool(name="ps_prod", bufs=1, space="PSUM"))
            ps_red = ctx.enter_context(tc.tile_pool(name="ps_red", bufs=1, space="PSUM"))

            # ---- load inputs
            t_pc = sing.tile([P, C, 9], F32)
            nc.sync.dma_start(t_pc[:], pc.rearrange("(p c) f -> p c f", p=P))
            t_w = sing.tile([P, 192], F16)
            nc.gpsimd.dma_start(t_w[:], w_in[:])
            t_id = sing.tile([P, P], F16)
            nc.gpsimd.dma_start(t_id[:], ident[:])

            # ---- features F32i (P, C, 32) f16:
            # slots [0:8]=hi of [x,y,z,x2,y2,z2,1,0], [8:16]=lo, [16:24]=hi dup,
            # [24:32]=0
            f32i = fpool.tile([P, C, 32], F16)
            sq32 = fpool.tile([P, C, 3], F32)
            nc.scalar.copy(f32i[:, :, 0:3], t_pc[:, :, 0:3])
            nc.scalar.square(sq32[:], t_pc[:, :, 0:3])
            nc.scalar.copy(f32i[:, :, 3:6], sq32[:])
            nc.gpsimd.memset(f32i[:, :, 6], 1.0)
            nc.gpsimd.memset(f32i[:, :, 7], 0.0)
            nc.vector.tensor_tensor(out=f32i[:, :, 8:11], in0=t_pc[:, :, 0:3],
                                    in1=f32i[:, :, 0:3], op=OP.subtract)
            nc.vector.tensor_tensor(out=f32i[:, :, 11:14], in0=sq32[:],
                                    in1=f32i[:, :, 3:6], op=OP.subtract)
            nc.gpsimd.memset(f32i[:, :, 14:16], 0.0)
            nc.vector.tensor_copy(f32i[:, :, 16:24], f32i[:, :, 0:8])
            nc.gpsimd.memset(f32i[:, :, 24:32], 0.0)

            # ---- persistent reduction accumulator
            red_ps = ps_red.tile([33, 128], F32)
            first_mm = True

            for half in range(2):
                nhc = 256  # chunks per half
                vals = half_pool.tile([P, 4, nhc, 32], F16, tag="vals")
                insv = half_pool.tile([P, nhc, 34], F16, tag="ins")
                sig32 = half_pool.tile([P, nhc, 32], F32, tag="sig")
                nc.gpsimd.memset(vals[:, 3], 1.0)
                nc.gpsimd.memset(insv[:, :, 32], 1.0)
                nc.gpsimd.memset(insv[:, :, 33], 0.0)

                for g in range(4):  # 4 groups x 64 chunks per half
                    rt_t = rt_pool.tile([P, 64, 96], F16, tag="rt")
                    mp_t = mp_pool.tile([P, 64, 32], F16, tag="mp")
                    for tt in range(16):
                        t = half * 64 + g * 16 + tt
                        ftb = ftb_pool.tile([P, P], F16, tag="ftb")
                        p_tr = ps_tr.tile([P, P], F16, tag="tr")
                        nc.tensor.transpose(p_tr[:], f32i[:, 4 * t:4 * (t + 1), :],
                                            t_id[:])
                        nc.vector.tensor_copy(ftb[:], p_tr[:])

                        quad = ps_prod.tile([P, 4, 512], F32, tag="prod")
                        for a in range(4):
                            nc.tensor.matmul(quad[:, a, 0:192],
                                             ftb[32 * a:32 * (a + 1), :],
                                             t_w[32 * a:32 * (a + 1), :],
                                             start=True, stop=True,
                                             tile_position=(32 * a, 0))
                        s0 = tt * 4
                        # ACT relu drain (true side, 4 chunks)
                        relu_in = bass.AP(tensor=quad.tensor, offset=quad.offset,
                                          ap=[quad.ap[0], [512, 4], [1, 96]])
                        nc.scalar.activation(rt_t[:, s0:s0 + 4, :], relu_in, AF.Relu)
                        # DVE min-pool drain (pred side; W prescaled by -100)
                        pool_in = bass.AP(tensor=quad.tensor, offset=quad.offset + 96,
                                          ap=[quad.ap[0], [512, 4], [1, 32], [32, 3]])
                        nc.vector.tensor_reduce(out=mp_t[:, s0:s0 + 4, :], in_=pool_in,
                                                axis=mybir.AxisListType.X, op=OP.min)

                    # ---- group postprocessing (64 chunks)
                    c0 = g * 64
                    sa = sa_pool.tile([P, 64, 32], F16, tag="sa")
                    nc.vector.tensor_tensor(out=sa[:], in0=rt_t[:, :, 0:32],
                                            in1=rt_t[:, :, 32:64], op=OP.add)
                    # inside = (sa + rz <= 0)  <=>  (-sa >= rz)
                    nc.vector.scalar_tensor_tensor(
                        out=insv[:, c0:c0 + 64, 0:32], in0=sa[:], scalar=-1.0,
                        in1=rt_t[:, :, 64:96], op0=OP.mult, op1=OP.is_ge)
                    # logits = clip(min3, -20, 20)
                    lg = sa_pool.tile([P, 64, 32], F16, tag="lg")
                    nc.vector.tensor_scalar(out=lg[:], in0=mp_t[:], scalar1=-20.0,
                                            scalar2=20.0, op0=OP.max, op1=OP.min)
                    nc.vector.tensor_copy(vals[:, 0, c0:c0 + 64, :], lg[:])
                    nc.scalar.activation(sig32[:, c0:c0 + 64, :], lg[:], AF.Sigmoid)
                    nc.vector.tensor_copy(vals[:, 1, c0:c0 + 64, :], sig32[:, c0:c0 + 64, :])

                # ---- per half: ln(sigmoid) (one table-set switch), then reductions
                nc.scalar.activation(vals[:, 2], sig32[:], AF.Ln)
                if dbg:
                    nc.gpsimd.dma_start(dbg_vals[half], vals[:])
                    nc.gpsimd.dma_start(dbg_ins[half], insv[:])
                for j in range(nhc):
                    lhsT = insv[:, j, 0:33]
                    rhs = bass.AP(tensor=vals.tensor, offset=vals.offset + j * 32,
                                  ap=[vals.ap[0], [nhc * 32, 4], [1, 32]])
                    nc.tensor.matmul(red_ps[:], lhsT, rhs,
                                     start=first_mm, stop=(half == 1 and j == nhc - 1))
                    first_mm = False

            s_red = sing.tile([33, 128], F32)
            nc.vector.tensor_copy(s_red[:], red_ps[:])
            nc.sync.dma_start(red_out[:], s_red[:])

    nc.compile()
    return nc


# ---------------------------------------------------------------- host helpers
def _host_w(vt_b, vp_b):
    """(128, 192) fp16 coefficient tile for one batch.

    Column j = side*96 + coord*32 + h (side 0 = true x256, 1 = pred x-100).
    K rows within each 32-row strip: [W_hi(8) | W_hi(8) | W_lo(8) | 0(8)],
    pairing with features [F_hi | F_lo | F_hi | 0]. Feature order:
    [x,y,z,x2,y2,z2,1,pad].
    """
    w8 = np.zeros((8, 192), dtype=np.float64)
    for side, (v, scale) in enumerate([(vt_b, TRUE_SCALE), (vp_b, PRED_SCALE)]):
        v1 = v[:, 0, :].astype(np.float64)
        v2 = v[:, 1, :].astype(np.float64)
        for c in range(3):
            base = side * 96 + c * 32
            w8[c, base:base + 32] = -(v1[:, c] + v2[:, c]) * scale
            w8[3 + c, base:base + 32] = 1.0 * scale
            w8[6, base:base + 32] = v1[:, c] * v2[:, c] * scale
    whi = w8.astype(np.float16)
    wlo = (w8 - whi.astype(np.float64)).astype(np.float16)
    stack = np.concatenate([whi, whi, wlo, np.zeros((8, 192), np.float16)], axis=0)
    return np.tile(stack, (4, 1))  # (128, 192)


def _lsa(cost):
    """Jonker-Volgenant shortest augmenting path; matches scipy col_ind."""
    cost = np.asarray(cost, dtype=np.float64)
    n = cost.shape[0]
    INF = np.inf
    u = np.zeros(n + 1)
    v = np.zeros(n + 1)
    p = np.zeros(n + 1, dtype=np.int64)
    way = np.zeros(n + 1, dtype=np.int64)
    for i in range(1, n + 1):
        p[0] = i
        j0 = 0
        minv = np.full(n + 1, INF)
        used = np.zeros(n + 1, dtype=bool)
        while True:
            used[j0] = True
            i0 = p[j0]
            cur = cost[i0 - 1, :] - u[i0] - v[1:]
            upd = (~used[1:]) & (cur < minv[1:])
            minv[1:][upd] = cur[upd]
            way[1:][upd] = j0
            free = ~used[1:]
            j1 = int(np.argmin(np.where(free, minv[1:], INF))) + 1
            delta = minv[j1]
            u[p[used]] += delta
            v[used] -= delta
            minv[~used] -= delta
            j0 = j1
            if p[j0] == 0:
                break
        while j0:
            j1 = way[j0]
            p[j0] = p[j1]
            j0 = j1
    ans = np.zeros(n, dtype=np.int64)
    for j in range(1, n + 1):
        ans[p[j] - 1] = j - 1
    return ans


def assemble_score(red, vertices_true, vertices_pred):
    """red: (B, 33, 128) device outputs -> score (B, 32, 32) float32."""
    score = np.zeros((B, H, H), dtype=np.float64)
    vt = np.asarray(vertices_true, dtype=np.float64)
    vp = np.asarray(vertices_pred, dtype=np.float64)
    for b in range(B):
        r = red[b].astype(np.float64)
        cross_n = r[0:32, 0:32]          # sum_n inside*l
        tp = r[0:32, 32:64]
        sum_inside = r[0:32, 96]
        sum_l = r[32, 0:32]
        sum_prob = r[32, 32:64]
        sum_ln = r[32, 64:96]
        sum_sp = sum_l - sum_ln          # softplus(l) = l - ln(sigmoid(l))
        bce = sum_sp[None, :] / N - cross_n / N
        denom = sum_prob[None, :] + sum_inside[:, None] - tp
        with np.errstate(divide="ignore", invalid="ignore"):
            iou = -np.where(denom == 0.0, 0.0, tp / denom)
        mse = np.mean((vt[b][:, None] - vp[b][None]) ** 2, axis=(2, 3))
        score[b] = bce + iou + mse
    return score.astype(np.float32)


def hungarian(score, vertices_true):
    lm = np.asarray(score)
    vt = np.asarray(vertices_true)
    mask = np.sum(np.abs(vt), axis=(2, 3)) == 0.0
    max_loss = 2.0 * lm.max()
    lm = np.where(mask[:, :, None], max_loss, lm)
    return np.stack([_lsa(lm[b]) for b in range(lm.shape[0])], axis=0).astype(np.int32)


def make_in_maps(pointcloud, vertices_true, vertices_pred):
    ident = np.eye(P, dtype=np.float16)
    in_maps = []
    for b in range(B):
        w = _host_w(np.asarray(vertices_true)[b], np.asarray(vertices_pred)[b])
        in_maps.append({
            "pc": np.ascontiguousarray(np.asarray(pointcloud)[b], dtype=np.float32),
            "w": w, "ident": ident,
        })
    return in_maps


def get_nc():
    if "nc" not in _NC_CACHE:
        _NC_CACHE["nc"] = build_nc()
    return _NC_CACHE["nc"]


def run_device(pointcloud, vertices_true, vertices_pred, trace=False):
    nc = get_nc()
    in_maps = make_in_maps(pointcloud, vertices_true, vertices_pred)
    res = run_bass_kernel_spmd(nc, in_maps, core_ids=list(range(B)), trace=trace)
    red = np.stack([res.results[i]["red"] for i in range(B)], axis=0)
    return red, res


def kernel(pointcloud, vertices_true, vertices_pred):
    red, _ = run_device(pointcloud, vertices_true, vertices_pred, trace=False)
    score = assemble_score(red, vertices_true, vertices_pred)
    return hungarian(score, vertices_true)
